# revision 11
# baseline (speedup 1.0000x reference)
# Trainium2 Bass kernel for nn_BinLinearEval:
#   out[b, o] = (round(x @ W.T + bias) * sign >= 0) ? 1.0 : 0.0
#
# Math folding (exact because bias is integer-valued and sign in {-1,+1}):
#   out = 1  iff  sign*(dot + bias) >= -0.5
#       = 1  iff  dot' >= thr_o      where dot' = x @ (sign.T*W).T  (W' still
#         ternary) and thr_o = -sign_o*bias_o - 0.5.
#
# Precision: x is shipped as an e4m3 hi + e4m3 residual*64 pair (2 B/elem,
# same HBM bytes as fp16) and BOTH passes run as fp8 DoubleRow matmuls at
# 0.5 cycles/column - the PE stream is ~2x faster than the fp16 single-pass
# variant, which measured clock-throttled to ~2 GHz under a dense fp16 MM
# stream. Accuracy: ~1713 threshold flips of 16.7M (rel err ~0.0143 vs the
# 2e-2 gate; verified in fp64 emulation and stable because inputs and the
# accumulation order are deterministic).
#
# PE schedule: groups of 512 batch columns; blocks of up to 4 groups share
# each DoubleRow LDWEIGHTS (256-col loads at ~213 ns would otherwise pace
# the stream); all 8 PSUM banks hold the block's accumulators. Block sizes
# ramp [1,1,2,4,...] so the first matmul gates on 0.5 MB of DMA, not 4 MB.
#
# DMA: x groups split across BOTH HWDGE rings (SP + ACT); weight tensor is
# DMA'd in hi/lo halves so the first matmul gates on 0.25 MB of weights.
# Output is the is_ge threshold emitted as fp8 (1.0/0.0 exact), 1 B/elem.

import os
from contextlib import ExitStack

import numpy as np
import ml_dtypes

BATCH, IN_F, OUT_F = 65536, 1024, 256
N_CORES = 8
B_CORE = BATCH // N_CORES  # 8192
P = 128
KC = IN_F // P             # 8 k-chunks of 128
NCH = KC                   # 8 DoubleRow chunk-steps: 4 hi + 4 lo, 256-contract each
OC = OUT_F // P            # 2 out-channel chunks
GRP = 512                  # batch tile / group size (= max matmul moving dim)
N_GROUPS = B_CORE // GRP   # 16
# ramp up so the first matmul gates on little DMA, taper down so the last
# epilogues + out-DMA drain while the PE is still busy
BLOCKS = [1, 1, 2, 4, 4, 2, 1, 1]
assert sum(BLOCKS) == N_GROUPS
# ring split: ~9 MB of x on the SP ring; ACT ring carries 7 MB of x plus
# weights up front and 2 MB of out-DMAs throughout
SYNC_GROUPS = frozenset([0, 2, 4, 6, 8, 10, 12, 14, 15])

_CACHE = {}


def _build():
    """Build (and cache) the Bass module. Returns the compiled nc."""
    if "nc" in _CACHE:
        return _CACHE["nc"]

    import concourse.bacc as bacc
    import concourse.mybir as mybir
    import concourse.tile as tile

    nc = bacc.Bacc(
        "TRN2",
        target_bir_lowering=False,
        debug=False,
        num_devices=N_CORES,
    )

    f32 = mybir.dt.float32
    f8 = mybir.dt.float8e4
    DR = mybir.MatmulPerfMode.DoubleRow

    # x8 chunk layout: [P, group, chunk(0:4 hi, 4:8 lo), j, GRP] where the
    # DoubleRow pair (chunk c, j) covers global k = (c%4)*256 + j*128 + p
    x8_d = nc.dram_tensor(
        "x8", [P, N_GROUPS, NCH, 2, GRP], f8, kind="ExternalInput"
    ).ap()
    w8_d = nc.dram_tensor("w8", [P, NCH, 2, OUT_F], f8, kind="ExternalInput").ap()
    thr_d = nc.dram_tensor("thr", [P, OC], f32, kind="ExternalInput").ap()
    out_d = nc.dram_tensor("out", [OC, P, B_CORE], f8, kind="ExternalOutput").ap()

    with tile.TileContext(nc) as tc, ExitStack() as ctx:
        const = ctx.enter_context(tc.tile_pool(name="const", bufs=1))
        io = ctx.enter_context(tc.tile_pool(name="io", bufs=14))
        outp = ctx.enter_context(tc.tile_pool(name="outp", bufs=4))
        psum = ctx.enter_context(tc.tile_pool(name="psum", bufs=8, space="PSUM"))

        # split the critical startup DMAs across both rings: weights (hi
        # half first) on SP, group 0's hi half on ACT -- the first
        # chunk-steps gate on 0.25 MB + 0.5 MB arriving in parallel
        w8_sb = const.tile([P, NCH, 2, OUT_F], f8)
        thr_sb = const.tile([P, OC], f32)
        tiles = {}
        xg0 = io.tile([P, NCH, 2, GRP], f8, name="xg0", bufs=1)
        tiles[0] = xg0
        nc.sync.dma_start(out=w8_sb[:, : NCH // 2], in_=w8_d[:, : NCH // 2])
        nc.scalar.dma_start(out=xg0[:, : NCH // 2], in_=x8_d[:, 0, : NCH // 2])
        nc.sync.dma_start(out=w8_sb[:, NCH // 2 :], in_=w8_d[:, NCH // 2 :])
        nc.scalar.dma_start(out=xg0[:, NCH // 2 :], in_=x8_d[:, 0, NCH // 2 :])
        nc.sync.dma_start(out=thr_sb, in_=thr_d)

        def issue(g):
            if g >= N_GROUPS or g in tiles:
                return
            eng = nc.sync if g in SYNC_GROUPS else nc.scalar
            t = io.tile([P, NCH, 2, GRP], f8, name="xg")
            eng.dma_start(out=t, in_=x8_d[:, g])
            tiles[g] = t

        blocks = []
        g0 = 0
        for b in BLOCKS:
            blocks.append(list(range(g0, g0 + b)))
            g0 += b

        for g in blocks[0] + blocks[1] + blocks[2]:
            issue(g)

        for bi, blk in enumerate(blocks):
            # 2-block prefetch lookahead keeps both rings streaming even
            # while out-DMAs wait on their epilogues
            for bj in (bi + 1, bi + 2):
                if bj < len(blocks):
                    for g in blocks[bj]:
                        issue(g)
            for oc in range(OC):
                pss = [psum.tile([P, GRP], f32, name="ps") for _ in blk]
                for c in range(NCH):
                    lhsT = w8_sb[:, c, :, oc * P : (oc + 1) * P]
                    for j, g in enumerate(blk):
                        nc.tensor.matmul(
                            pss[j],
                            lhsT,
                            tiles[g][:, c],
                            start=(c == 0),
                            stop=(c == NCH - 1),
                            perf_mode=DR,
                        )
                # one fat out-DMA per (block, oc): 2 KB DRAM lines instead
                # of 512 B, and 4x fewer descriptors on the ACT ring
                ob = outp.tile([P, len(blk) * GRP], f8, name=f"ob{len(blk)}")
                for j, g in enumerate(blk):
                    nc.vector.tensor_scalar(
                        ob[:, j * GRP : (j + 1) * GRP],
                        pss[j],
                        thr_sb[:, oc : oc + 1],
                        None,
                        mybir.AluOpType.is_ge,
                    )
                nc.scalar.dma_start(
                    out=out_d[oc, :, blk[0] * GRP : (blk[-1] + 1) * GRP], in_=ob
                )
            for g in blk:
                tiles.pop(g)

    nc.compile()
    _CACHE["nc"] = nc
    return nc


def _prep_inputs(x, weight, bias, sign):
    """Host-side prep: fold sign into weights, build thresholds, split x into
    an e4m3 hi + e4m3 residual*64 pair in DoubleRow-interleaved layout."""
    f8np = ml_dtypes.float8_e4m3fn
    x = np.asarray(x, dtype=np.float32)
    weight = np.asarray(weight, dtype=np.float32)
    bias = np.asarray(bias, dtype=np.float32)
    sign = np.asarray(sign, dtype=np.float32).reshape(1, OUT_F)

    wp = sign.T * weight                      # [OUT_F, IN_F], ternary
    thr = (-sign[0] * bias - np.float32(0.5)).astype(np.float32)  # [OUT_F]
    thr2 = np.ascontiguousarray(thr.reshape(OC, P).T)  # [P, OC]

    # weights: [P, chunk, j, OUT_F]; chunks 0:4 = W' (ternary, exact in
    # e4m3), 4:8 = W'/64 (+-2^-6, exact in e4m3)
    wT = wp.T  # [IN_F, OUT_F]
    whi = wT.reshape(NCH // 2, 2, P, OUT_F).transpose(2, 0, 1, 3)
    wlo = (wT * np.float32(1.0 / 64.0)).reshape(NCH // 2, 2, P, OUT_F).transpose(
        2, 0, 1, 3
    )
    w8 = np.ascontiguousarray(
        np.concatenate([whi, wlo], axis=1)
    ).astype(f8np)                            # [P, NCH, 2, OUT_F]

    xhi8 = x.astype(f8np)
    xlo8 = ((x - xhi8.astype(np.float32)) * np.float32(64.0)).astype(f8np)

    in_maps = []
    for c in range(N_CORES):
        sl = slice(c * B_CORE, (c + 1) * B_CORE)
        hi = xhi8[sl].reshape(N_GROUPS, GRP, NCH // 2, 2, P).transpose(
            4, 0, 2, 3, 1
        )                                      # [P, g, 4, 2, GRP]
        lo = xlo8[sl].reshape(N_GROUPS, GRP, NCH // 2, 2, P).transpose(
            4, 0, 2, 3, 1
        )
        x8 = np.ascontiguousarray(np.concatenate([hi, lo], axis=2))
        in_maps.append({"x8": x8, "w8": w8, "thr": thr2})
    return in_maps


def _assemble(results):
    """[core][OC, P, B_CORE] fp8 -> [BATCH, OUT_F] fp32"""
    full = np.concatenate(
        [
            np.asarray(r["out"])
            .view(ml_dtypes.float8_e4m3fn)
            .astype(np.float32)
            .reshape(OUT_F, B_CORE)
            for r in results
        ],
        axis=1,
    )  # [OUT_F, BATCH]
    return np.ascontiguousarray(full.T)


def run(x, weight, bias, sign, trace=False):
    """Run the kernel; returns (output, BassKernelResults)."""
    from concourse.bass_utils import run_bass_kernel_spmd

    if not trace:
        os.environ["BASS_NEVER_TRACE"] = "1"
    else:
        os.environ.pop("BASS_NEVER_TRACE", None)

    nc = _build()
    in_maps = _prep_inputs(x, weight, bias, sign)
    res = run_bass_kernel_spmd(
        nc,
        in_maps,
        core_ids=list(range(N_CORES)),
        trace=trace,
    )
    return _assemble(res.results), res


def kernel(x, weight, bias, sign):
    out, _ = run(x, weight, bias, sign, trace=False)
    return out


# revision 12
# speedup vs baseline: 1.0718x; 1.0718x over previous
# Trainium2 Bass kernel for nn_BinLinearEval:
#   out[b, o] = (round(x @ W.T + bias) * sign >= 0) ? 1.0 : 0.0
#
# Math folding (exact because bias is integer-valued and sign in {-1,+1}):
#   out = 1  iff  sign*(dot + bias) >= -0.5
#       = 1  iff  dot' >= thr_o      where dot' = x @ (sign.T*W).T  (W' still
#         ternary) and thr_o = -sign_o*bias_o - 0.5.
#
# Precision: x is shipped as an e4m3 hi + e4m3 residual*64 pair (2 B/elem,
# same HBM bytes as fp16) and BOTH passes run as fp8 DoubleRow matmuls at
# 0.5 cycles/column - the PE stream is ~2x faster than the fp16 single-pass
# variant, which measured clock-throttled to ~2 GHz under a dense fp16 MM
# stream. Accuracy: ~1713 threshold flips of 16.7M (rel err ~0.0143 vs the
# 2e-2 gate; verified in fp64 emulation and stable because inputs and the
# accumulation order are deterministic).
#
# PE schedule: groups of 512 batch columns; blocks of up to 4 groups share
# each DoubleRow LDWEIGHTS (256-col loads at ~213 ns would otherwise pace
# the stream); all 8 PSUM banks hold the block's accumulators. Block sizes
# ramp [1,1,2,4,...] so the first matmul gates on 0.5 MB of DMA, not 4 MB.
#
# DMA: x groups split across BOTH HWDGE rings (SP + ACT); weight tensor is
# DMA'd in hi/lo halves so the first matmul gates on 0.25 MB of weights.
# Output is the is_ge threshold emitted as fp8 (1.0/0.0 exact), 1 B/elem.

import os
from contextlib import ExitStack

import numpy as np
import ml_dtypes

BATCH, IN_F, OUT_F = 65536, 1024, 256
N_CORES = 8
B_CORE = BATCH // N_CORES  # 8192
P = 128
KC = IN_F // P             # 8 k-chunks of 128
NCH = KC                   # 8 DoubleRow chunk-steps: 4 hi + 4 lo, 256-contract each
OC = OUT_F // P            # 2 out-channel chunks
GRP = 512                  # batch tile / group size (= max matmul moving dim)
N_GROUPS = B_CORE // GRP   # 16
# ramp up so the first matmul gates on little DMA, taper down so the last
# epilogues + out-DMA drain while the PE is still busy
BLOCKS = [1, 1, 2, 4, 4, 2, 1, 1]
assert sum(BLOCKS) == N_GROUPS
# ring split: ~9 MB of x on the SP ring; ACT ring carries 7 MB of x plus
# weights up front and 2 MB of out-DMAs throughout
SYNC_GROUPS = frozenset([0, 2, 4, 6, 8, 10, 12, 14, 15])

_CACHE = {}


def _build():
    """Build (and cache) the Bass module. Returns the compiled nc."""
    if "nc" in _CACHE:
        return _CACHE["nc"]

    import concourse.bacc as bacc
    import concourse.mybir as mybir
    import concourse.tile as tile

    nc = bacc.Bacc(
        "TRN2",
        target_bir_lowering=False,
        debug=False,
        num_devices=N_CORES,
    )

    f32 = mybir.dt.float32
    f8 = mybir.dt.float8e4
    DR = mybir.MatmulPerfMode.DoubleRow

    # x8 chunk layout: [P, group, chunk(0:4 hi, 4:8 lo), j, GRP] where the
    # DoubleRow pair (chunk c, j) covers global k = (c%4)*256 + j*128 + p
    x8_d = nc.dram_tensor(
        "x8", [P, N_GROUPS, NCH, 2, GRP], f8, kind="ExternalInput"
    ).ap()
    w8_d = nc.dram_tensor("w8", [P, NCH, 2, OUT_F], f8, kind="ExternalInput").ap()
    thr_d = nc.dram_tensor("thr", [P, OC], f32, kind="ExternalInput").ap()
    out_d = nc.dram_tensor("out", [OC, P, B_CORE], f8, kind="ExternalOutput").ap()

    with tile.TileContext(nc) as tc, ExitStack() as ctx:
        const = ctx.enter_context(tc.tile_pool(name="const", bufs=1))
        io = ctx.enter_context(tc.tile_pool(name="io", bufs=12))
        outp = ctx.enter_context(tc.tile_pool(name="outp", bufs=4))
        psum = ctx.enter_context(tc.tile_pool(name="psum", bufs=8, space="PSUM"))

        # split the critical startup DMAs across both rings: weights (hi
        # half first) on SP, group 0's hi half on ACT -- the first
        # chunk-steps gate on 0.25 MB + 0.5 MB arriving in parallel
        w8_sb = const.tile([P, NCH, 2, OUT_F], f8)
        thr_sb = const.tile([P, OC], f32)
        tiles = {}
        xg0 = io.tile([P, NCH, 2, GRP], f8, name="xg0", bufs=1)
        tiles[0] = xg0
        nc.sync.dma_start(out=w8_sb[:, : NCH // 2], in_=w8_d[:, : NCH // 2])
        nc.scalar.dma_start(out=xg0[:, : NCH // 2], in_=x8_d[:, 0, : NCH // 2])
        nc.sync.dma_start(out=w8_sb[:, NCH // 2 :], in_=w8_d[:, NCH // 2 :])
        nc.scalar.dma_start(out=xg0[:, NCH // 2 :], in_=x8_d[:, 0, NCH // 2 :])
        nc.sync.dma_start(out=thr_sb, in_=thr_d)

        def issue(g):
            if g >= N_GROUPS or g in tiles:
                return
            eng = nc.sync if g in SYNC_GROUPS else nc.scalar
            t = io.tile([P, NCH, 2, GRP], f8, name="xg")
            eng.dma_start(out=t, in_=x8_d[:, g])
            tiles[g] = t

        blocks = []
        g0 = 0
        for b in BLOCKS:
            blocks.append(list(range(g0, g0 + b)))
            g0 += b

        for g in blocks[0] + blocks[1] + blocks[2]:
            issue(g)

        for bi, blk in enumerate(blocks):
            # 2-block prefetch lookahead keeps both rings streaming even
            # while out-DMAs wait on their epilogues
            for bj in (bi + 1, bi + 2):
                if bj < len(blocks):
                    for g in blocks[bj]:
                        issue(g)
            for oc in range(OC):
                pss = [psum.tile([P, GRP], f32, name="ps") for _ in blk]
                for c in range(NCH):
                    lhsT = w8_sb[:, c, :, oc * P : (oc + 1) * P]
                    for j, g in enumerate(blk):
                        nc.tensor.matmul(
                            pss[j],
                            lhsT,
                            tiles[g][:, c],
                            start=(c == 0),
                            stop=(c == NCH - 1),
                            perf_mode=DR,
                        )
                # one fat out-DMA per (block, oc): 2 KB DRAM lines instead
                # of 512 B, and 4x fewer descriptors on the ACT ring
                ob = outp.tile([P, len(blk) * GRP], f8, name=f"ob{len(blk)}")
                for j, g in enumerate(blk):
                    nc.vector.tensor_scalar(
                        ob[:, j * GRP : (j + 1) * GRP],
                        pss[j],
                        thr_sb[:, oc : oc + 1],
                        None,
                        mybir.AluOpType.is_ge,
                    )
                nc.scalar.dma_start(
                    out=out_d[oc, :, blk[0] * GRP : (blk[-1] + 1) * GRP], in_=ob
                )
            for g in blk:
                tiles.pop(g)

    nc.compile()
    _CACHE["nc"] = nc
    return nc


def _prep_inputs(x, weight, bias, sign):
    """Host-side prep: fold sign into weights, build thresholds, split x into
    an e4m3 hi + e4m3 residual*64 pair in DoubleRow-interleaved layout."""
    f8np = ml_dtypes.float8_e4m3fn
    x = np.asarray(x, dtype=np.float32)
    weight = np.asarray(weight, dtype=np.float32)
    bias = np.asarray(bias, dtype=np.float32)
    sign = np.asarray(sign, dtype=np.float32).reshape(1, OUT_F)

    wp = sign.T * weight                      # [OUT_F, IN_F], ternary
    thr = (-sign[0] * bias - np.float32(0.5)).astype(np.float32)  # [OUT_F]
    thr2 = np.ascontiguousarray(thr.reshape(OC, P).T)  # [P, OC]

    # weights: [P, chunk, j, OUT_F]; chunks 0:4 = W' (ternary, exact in
    # e4m3), 4:8 = W'/64 (+-2^-6, exact in e4m3)
    wT = wp.T  # [IN_F, OUT_F]
    whi = wT.reshape(NCH // 2, 2, P, OUT_F).transpose(2, 0, 1, 3)
    wlo = (wT * np.float32(1.0 / 64.0)).reshape(NCH // 2, 2, P, OUT_F).transpose(
        2, 0, 1, 3
    )
    w8 = np.ascontiguousarray(
        np.concatenate([whi, wlo], axis=1)
    ).astype(f8np)                            # [P, NCH, 2, OUT_F]

    xhi8 = x.astype(f8np)
    xlo8 = ((x - xhi8.astype(np.float32)) * np.float32(64.0)).astype(f8np)

    in_maps = []
    for c in range(N_CORES):
        sl = slice(c * B_CORE, (c + 1) * B_CORE)
        hi = xhi8[sl].reshape(N_GROUPS, GRP, NCH // 2, 2, P).transpose(
            4, 0, 2, 3, 1
        )                                      # [P, g, 4, 2, GRP]
        lo = xlo8[sl].reshape(N_GROUPS, GRP, NCH // 2, 2, P).transpose(
            4, 0, 2, 3, 1
        )
        x8 = np.ascontiguousarray(np.concatenate([hi, lo], axis=2))
        in_maps.append({"x8": x8, "w8": w8, "thr": thr2})
    return in_maps


def _assemble(results):
    """[core][OC, P, B_CORE] fp8 -> [BATCH, OUT_F] fp32"""
    full = np.concatenate(
        [
            np.asarray(r["out"])
            .view(ml_dtypes.float8_e4m3fn)
            .astype(np.float32)
            .reshape(OUT_F, B_CORE)
            for r in results
        ],
        axis=1,
    )  # [OUT_F, BATCH]
    return np.ascontiguousarray(full.T)


def run(x, weight, bias, sign, trace=False):
    """Run the kernel; returns (output, BassKernelResults)."""
    from concourse.bass_utils import run_bass_kernel_spmd

    if not trace:
        os.environ["BASS_NEVER_TRACE"] = "1"
    else:
        os.environ.pop("BASS_NEVER_TRACE", None)

    nc = _build()
    in_maps = _prep_inputs(x, weight, bias, sign)
    res = run_bass_kernel_spmd(
        nc,
        in_maps,
        core_ids=list(range(N_CORES)),
        trace=trace,
    )
    return _assemble(res.results), res


def kernel(x, weight, bias, sign):
    out, _ = run(x, weight, bias, sign, trace=False)
    return out
